# revision 10
# baseline (speedup 1.0000x reference)
"""BiLSTM-CRF kernel for Trainium2 (8 NeuronCores, SPMD).

Sharding: 8 cores x (8 sequences, ONE LSTM direction each). Cores 0-3 run
the forward LSTM on sequences 8c..8c+7; cores 4-7 run the backward LSTM as
a forward scan over time-reversed inputs of the same sequences. Each core
computes, fully on device:
  1. input projection xg = [x|1] @ [Wih|b].T  (chunked GEMM, overlapped
     with the scan),
  2. the 512-step LSTM recurrence (TensorE h@Whh.T, ScalarE sigmoid/tanh,
     VectorE gate arithmetic),
  3. its direction's partial emissions h @ W_out_half.T  -> [4096, 4].
Host does the embedding gather, sums fwd/bwd emission partials (+b_out),
and runs exact fp32 Viterbi + masking.

The Bass program is built/compiled once at import (with a dummy warmup run
so the NEFF is loaded and the axon session established); kernel() then
only pays a steady-state device execution, which is what LAST_DEVICE_NS
reports.
"""

import sys
import time

for _p in ("/opt/trn_rl_repo", "/root/.axon_site/_ro/trn_rl_repo"):
    if _p not in sys.path:
        sys.path.insert(0, _p)

import numpy as np

B, L, V, E, H, T = 32, 512, 100000, 300, 256, 4
NCORES = 8
SEQ = 8                  # sequences per core
TOK = SEQ * L            # 4096 tokens per core
G = 4 * H                # 1024 gate rows
EA = 304                 # 300 features + bias row + pad to 8
KI = 3                   # K tiles for input projection (384 = 3*128)
NCHUNK = 4               # time chunks for the input-projection GEMM
TC = L // NCHUNK         # 128 steps per chunk

LAST_DEVICE_NS = None    # wall-time of the timed device execution
_STATE = {}

# row permutation: torch gate order (i,f,g,o) -> device order (i,f,o,g)
_PERM = np.r_[0:2 * H, 3 * H:4 * H, 2 * H:3 * H]


# ---------------------------------------------------------------- device IR
def _build_nc():
    import concourse.bacc as bacc
    import concourse.mybir as mybir
    from concourse.tile import TileContext

    f32 = mybir.dt.float32
    f32r = mybir.dt.float32r
    SIG = mybir.ActivationFunctionType.Sigmoid
    TANH = mybir.ActivationFunctionType.Tanh

    nc = bacc.Bacc()
    xT_d = nc.declare_dram_parameter("xT", [EA, TOK], f32r, isOutput=False)
    wih_d = nc.declare_dram_parameter("wih", [KI, 128, G], f32r, isOutput=False)
    whh_d = nc.declare_dram_parameter("whh", [2, 128, G], f32r, isOutput=False)
    wo_d = nc.declare_dram_parameter("wo", [2, 128, T], f32r, isOutput=False)
    emis_d = nc.declare_dram_parameter("emis", [TOK, T], f32, isOutput=True)

    with TileContext(nc) as tc:
        const = tc.alloc_tile_pool(name="const", bufs=1)
        xtp = tc.alloc_tile_pool(name="xtp", bufs=2)
        xgp = tc.alloc_tile_pool(name="xgp", bufs=2)
        hsp = tc.alloc_tile_pool(name="hsp", bufs=1)
        sp = tc.alloc_tile_pool(name="sp", bufs=3)
        p1p = tc.alloc_tile_pool(name="p1p", bufs=2, space="PSUM")
        gpp = tc.alloc_tile_pool(name="gpp", bufs=2, space="PSUM")
        pep = tc.alloc_tile_pool(name="pep", bufs=2, space="PSUM")

        # ---- constants / weights
        wih_sb = const.tile([128, KI, G], f32r)
        nc.sync.dma_start(wih_sb[:], wih_d[:].rearrange("ko ki g -> ki ko g"))
        whh_sb = const.tile([128, 2, G], f32r)
        nc.sync.dma_start(whh_sb[:], whh_d[:].rearrange("k ki g -> ki k g"))
        wo_sb = const.tile([128, 2, T], f32r)
        nc.sync.dma_start(wo_sb[:], wo_d[:].rearrange("k ki t -> ki k t"))

        # ---- persistent state
        hs0 = hsp.tile([128, TOK], f32r, tag="hs0")   # h rows 0:128
        hs1 = hsp.tile([128, TOK], f32r, tag="hs1")   # h rows 128:256
        hs = [hs0, hs1]
        hs3 = [h.rearrange("p (s l) -> p s l", l=L) for h in hs]
        c_sb = hsp.tile([128, 2 * SEQ], f32, tag="c")

        xT3 = xT_d[:].rearrange("k (s l) -> k s l", s=SEQ)

        xg_tiles = []

        def phase1(cchunk):
            """Input projection for steps [cchunk*TC, (cchunk+1)*TC)."""
            xtc = xtp.tile([128, 2, SEQ, TC], f32r, tag="xtc")
            xtc2 = xtp.tile([EA - 256, SEQ, TC], f32r, tag="xtc2")
            tsl = slice(cchunk * TC, (cchunk + 1) * TC)
            for ko in range(2):
                src = xT_d[ko * 128:(ko + 1) * 128, :].rearrange(
                    "ki (s l) -> ki s l", s=SEQ)[:, :, tsl]
                nc.sync.dma_start(xtc[:, ko], src)
            src2 = xT_d[256:EA, :].rearrange(
                "ki (s l) -> ki s l", s=SEQ)[:, :, tsl]
            nc.sync.dma_start(xtc2[:], src2)

            xg = xgp.tile([128, TC, 8 * SEQ], f32, tag="xg")
            for m in range(8):
                for sh in range(2):
                    ps = p1p.tile([128, 512], f32, tag="p1")
                    for ko in range(2):
                        nc.tensor.matmul(
                            ps[:],
                            wih_sb[:, ko, m * 128:(m + 1) * 128],
                            xtc[:, ko, sh * 4:(sh + 1) * 4, :],
                            start=(ko == 0),
                            stop=False,
                        )
                    nc.tensor.matmul(
                        ps[:],
                        wih_sb[0:EA - 256, 2, m * 128:(m + 1) * 128],
                        xtc2[:, sh * 4:(sh + 1) * 4, :],
                        start=False,
                        stop=True,
                    )
                    dst = xg[:, :, m * 8 + sh * 4: m * 8 + sh * 4 + 4]
                    nc.scalar.copy(
                        dst.rearrange("p l s -> p s l"),
                        ps[:].rearrange("p (s l) -> p s l", s=4),
                    )
            xg_tiles.append(xg)

        def step(t):
            cchunk, tl = divmod(t, TC)
            gates = sp.tile([128, 8 * SEQ], f32, tag="gates")
            if t == 0:
                nc.vector.tensor_copy(gates[:], xg_tiles[0][:, 0, :])
            else:
                gp = gpp.tile([128, 8 * SEQ], f32, tag="gp")
                for m in range(8):
                    for k in range(2):
                        nc.tensor.matmul(
                            gp[:, m * SEQ:(m + 1) * SEQ],
                            whh_sb[:, k, m * 128:(m + 1) * 128],
                            hs3[k][:, :, t - 1],
                            start=(k == 0),
                            stop=(k == 1),
                        )
                nc.vector.tensor_add(gates[:], gp[:], xg_tiles[cchunk][:, tl, :])
            sg = sp.tile([128, 6 * SEQ], f32, tag="sg")
            nc.scalar.activation(sg[:], gates[:, 0:6 * SEQ], SIG)
            tg = sp.tile([128, 2 * SEQ], f32, tag="tg")
            nc.scalar.activation(tg[:], gates[:, 6 * SEQ:8 * SEQ], TANH)
            if t == 0:
                nc.vector.tensor_mul(c_sb[:], sg[:, 0:2 * SEQ], tg[:])
            else:
                ig = sp.tile([128, 2 * SEQ], f32, tag="ig")
                nc.vector.tensor_mul(ig[:], sg[:, 0:2 * SEQ], tg[:])
                fc = sp.tile([128, 2 * SEQ], f32, tag="fc")
                nc.vector.tensor_mul(fc[:], sg[:, 2 * SEQ:4 * SEQ], c_sb[:])
                nc.vector.tensor_add(c_sb[:], ig[:], fc[:])
            th = sp.tile([128, 2 * SEQ], f32, tag="th")
            nc.scalar.activation(th[:], c_sb[:], TANH)
            for k in range(2):
                nc.vector.tensor_mul(
                    hs3[k][:, :, t],
                    sg[:, (4 + k) * SEQ:(5 + k) * SEQ],
                    th[:, k * SEQ:(k + 1) * SEQ],
                )

        # interleave: projection of chunk c+1 is emitted before the scan of
        # chunk c so the PE fills its idle time during the recurrence.
        phase1(0)
        phase1(1)
        for t in range(TC):
            step(t)
        phase1(2)
        for t in range(TC, 2 * TC):
            step(t)
        phase1(3)
        for t in range(2 * TC, L):
            step(t)

        # ---- partial emissions: e = h @ W_out_half.T   [TOK, T]
        emis_sb = const.tile([128, TOK // 128, T], f32)
        for mt in range(TOK // 128):
            pe = pep.tile([128, T], f32, tag="pe")
            for k in range(2):
                nc.tensor.matmul(
                    pe[:],
                    hs[k][:, mt * 128:(mt + 1) * 128],
                    wo_sb[:, k, :],
                    start=(k == 0),
                    stop=(k == 1),
                )
            nc.vector.tensor_copy(emis_sb[:, mt, :], pe[:])
        nc.sync.dma_start(
            emis_d[:].rearrange("(mt p) t -> p mt t", p=128), emis_sb[:]
        )

        for pool in (pep, gpp, p1p, sp, hsp, xgp, xtp, const):
            pool.release()

    nc.finalize()
    return nc


# ---------------------------------------------------------------- runner
def _ensure_ready():
    """Build + jit-compile the sharded executable and warm it up once."""
    if "fn" in _STATE or _STATE.get("broken"):
        return
    try:
        import jax
        import jax.numpy as jnp  # noqa: F401
        from jax.sharding import Mesh, PartitionSpec
        from jax.experimental.shard_map import shard_map
        import concourse.mybir as mybir
        from concourse import bass2jax

        t0 = time.perf_counter()
        nc = _build_nc()
        print(f"[kernel] bass build: {time.perf_counter() - t0:.1f}s",
              file=sys.stderr, flush=True)

        bass2jax.install_neuronx_cc_hook()
        partition_name = (nc.partition_id_tensor.name
                          if nc.partition_id_tensor else None)
        in_names, out_names, out_avals, zero_outs = [], [], [], []
        for alloc in nc.m.functions[0].allocations:
            if not isinstance(alloc, mybir.MemoryLocationSet):
                continue
            name = alloc.memorylocations[0].name
            if alloc.kind == "ExternalInput":
                if name != partition_name:
                    in_names.append(name)
            elif alloc.kind == "ExternalOutput":
                out_names.append(name)
                shape = tuple(alloc.tensor_shape)
                dtype = mybir.dt.np(alloc.dtype)
                out_avals.append(jax.core.ShapedArray(shape, dtype))
                zero_outs.append(np.zeros(shape, dtype))
        n_params = len(in_names)
        all_in_names = in_names + out_names
        if partition_name is not None:
            all_in_names = all_in_names + [partition_name]

        def _body(*args):
            operands = list(args)
            if partition_name is not None:
                operands.append(bass2jax.partition_id_tensor())
            outs = bass2jax._bass_exec_p.bind(
                *operands,
                out_avals=tuple(out_avals),
                in_names=tuple(all_in_names),
                out_names=tuple(out_names),
                lowering_input_output_aliases=(),
                sim_require_finite=True,
                sim_require_nnan=True,
                nc=nc,
            )
            return tuple(outs)

        devices = jax.devices()[:NCORES]
        mesh = Mesh(np.asarray(devices), ("core",))
        n_outs = len(out_names)
        fn = jax.jit(
            shard_map(
                _body, mesh=mesh,
                in_specs=(PartitionSpec("core"),) * (n_params + n_outs),
                out_specs=(PartitionSpec("core"),) * n_outs,
                check_rep=False,
            ),
            donate_argnums=tuple(range(n_params, n_params + n_outs)),
            keep_unused=True,
        )

        _STATE["in_names"] = in_names
        _STATE["out_names"] = out_names
        _STATE["zero_outs"] = zero_outs
        _STATE["mesh"] = mesh

        # warm up: compiles the NEFF, loads it on all 8 cores.
        t0 = time.perf_counter()
        dummies = _stack_inputs({n: None for n in in_names})
        outs = fn(*dummies, *_concat_zeros())
        np.asarray(outs[0])
        print(f"[kernel] compile+warmup: {time.perf_counter() - t0:.1f}s",
              file=sys.stderr, flush=True)
        _STATE["fn"] = fn
    except Exception as exc:  # pragma: no cover - fallback safety
        print(f"[kernel] device path unavailable ({exc!r}); "
              "falling back to host numpy", file=sys.stderr, flush=True)
        _STATE["broken"] = True


_IN_SHAPES = {
    "xT": (EA, TOK), "wih": (KI, 128, G), "whh": (2, 128, G),
    "wo": (2, 128, T),
}


def _stack_inputs(per_core):
    """per_core: name -> list of 8 arrays (or None for zeros)."""
    out = []
    for name in _STATE["in_names"]:
        arrs = per_core.get(name)
        if arrs is None:
            out.append(np.zeros((NCORES,) + _IN_SHAPES[name], np.float32)
                       .reshape(NCORES * _IN_SHAPES[name][0],
                                *_IN_SHAPES[name][1:]))
        else:
            out.append(np.concatenate(arrs, axis=0))
    return out


def _concat_zeros():
    return [np.zeros((NCORES * z.shape[0],) + z.shape[1:], z.dtype)
            for z in _STATE["zero_outs"]]


def _pack_weights(Wih, b, Whh, Wo_half):
    """-> (wih [KI,128,G], whh [2,128,G], wo [2,128,T]) fp32."""
    Wih_p = Wih[_PERM]
    b_p = b[_PERM]
    Whh_p = Whh[_PERM]
    wih_aug = np.zeros((KI * 128, G), np.float32)
    wih_aug[:E] = Wih_p.T.astype(np.float32)
    wih_aug[E] = b_p.astype(np.float32)
    wih = wih_aug.reshape(KI, 128, G)
    whh = np.ascontiguousarray(Whh_p.T.astype(np.float32)).reshape(2, 128, G)
    wo = np.ascontiguousarray(Wo_half.T.astype(np.float32)).reshape(2, 128, T)
    return wih, whh, wo


def _make_xT(x_block):
    """x_block [SEQ, L, E] -> xT [EA, TOK] fp32 with bias row."""
    xt = np.empty((EA, TOK), np.float32)
    flat = x_block.reshape(TOK, E)
    xt[:E] = flat.T
    xt[E] = 1.0
    xt[E + 1:] = 0.0
    return xt


def _viterbi_host(emissions, mask, transitions, start_trans, end_trans):
    trans = np.asarray(transitions, np.float32)
    m = mask.astype(bool)
    score = np.asarray(start_trans, np.float32) + emissions[:, 0]
    history = np.empty((L - 1, B, T), np.int32)
    for t in range(1, L):
        cand = score[:, :, None] + trans[None] + emissions[:, t][:, None, :]
        history[t - 1] = np.argmax(cand, axis=1).astype(np.int32)
        new = np.max(cand, axis=1)
        score = np.where(m[:, t][:, None], new, score)
    score = score + np.asarray(end_trans, np.float32)
    last_tag = np.argmax(score, axis=-1).astype(np.int32)
    tags = np.empty((B, L), np.int32)
    tags[:, L - 1] = last_tag
    tag = last_tag
    rows = np.arange(B)
    for t in range(L - 2, -1, -1):
        prev = history[t][rows, tag]
        tag = np.where(m[:, t + 1], prev, tag).astype(np.int32)
        tags[:, t] = tag
    return tags


def _sigmoid(v):
    return np.float32(1.0) / (np.float32(1.0) + np.exp(-v))


def _host_emissions(x, Wih, Whh, b, reverse):
    """Exact fp32 fallback LSTM for one direction."""
    xs = x[:, ::-1] if reverse else x
    xg = xs @ Wih.T + b
    n = x.shape[0]
    h = np.zeros((n, H), np.float32)
    c = np.zeros((n, H), np.float32)
    WhhT = np.ascontiguousarray(Whh.T)
    hs = np.empty((L, n, H), np.float32)
    for t in range(L):
        g = xg[:, t] + h @ WhhT
        i = _sigmoid(g[:, :H])
        f = _sigmoid(g[:, H:2 * H])
        gg = np.tanh(g[:, 2 * H:3 * H])
        o = _sigmoid(g[:, 3 * H:])
        c = f * c + i * gg
        h = o * np.tanh(c)
        hs[t] = h
    hs = np.swapaxes(hs, 0, 1)
    return hs[:, ::-1] if reverse else hs


def kernel(word_ids, mask, label_ids, emb, Wih_f, Whh_f, b_f, Wih_b, Whh_b,
           b_b, W_out, b_out, transitions, start_trans, end_trans):
    global LAST_DEVICE_NS
    word_ids = np.asarray(word_ids, np.int32)
    mask = np.asarray(mask, np.int32)
    emb = np.asarray(emb, np.float32)
    W_out = np.asarray(W_out, np.float32)
    b_out = np.asarray(b_out, np.float32)

    x = emb[word_ids]  # [B, L, E] host gather

    _ensure_ready()
    if not _STATE.get("broken"):
        emissions = _device_emissions(x, Wih_f, Whh_f, b_f, Wih_b, Whh_b,
                                      b_b, W_out, b_out)
    else:
        h_f = _host_emissions(x, np.asarray(Wih_f, np.float32),
                              np.asarray(Whh_f, np.float32),
                              np.asarray(b_f, np.float32), False)
        h_b = _host_emissions(x, np.asarray(Wih_b, np.float32),
                              np.asarray(Whh_b, np.float32),
                              np.asarray(b_b, np.float32), True)
        hcat = np.concatenate([h_f, h_b], axis=-1)
        emissions = hcat @ W_out.T + b_out

    tags = _viterbi_host(emissions, mask, transitions, start_trans, end_trans)
    return (tags * mask).astype(np.int32)


def _device_emissions(x, Wih_f, Whh_f, b_f, Wih_b, Whh_b, b_b, W_out, b_out):
    global LAST_DEVICE_NS
    wih_f, whh_f, wo_f = _pack_weights(
        np.asarray(Wih_f, np.float32), np.asarray(b_f, np.float32),
        np.asarray(Whh_f, np.float32), W_out[:, :H])
    wih_b, whh_b, wo_b = _pack_weights(
        np.asarray(Wih_b, np.float32), np.asarray(b_b, np.float32),
        np.asarray(Whh_b, np.float32), W_out[:, H:])

    xts, wihs, whhs, wos = [], [], [], []
    for c in range(4):
        xts.append(_make_xT(x[c * SEQ:(c + 1) * SEQ]))
        wihs.append(wih_f); whhs.append(whh_f); wos.append(wo_f)
    for c in range(4):
        xts.append(_make_xT(x[c * SEQ:(c + 1) * SEQ][:, ::-1]))
        wihs.append(wih_b); whhs.append(whh_b); wos.append(wo_b)

    ins = _stack_inputs({"xT": xts, "wih": wihs, "whh": whhs, "wo": wos})
    fn = _STATE["fn"]

    # stage inputs + donated output buffers into HBM (not part of the timed
    # kernel execution, same as any kernel benchmark's H2D staging)
    import jax
    from jax.sharding import NamedSharding, PartitionSpec
    sh = NamedSharding(_STATE["mesh"], PartitionSpec("core"))
    t0 = time.perf_counter()
    staged = [jax.device_put(a, sh) for a in ins + _concat_zeros()]
    jax.block_until_ready(staged)
    stage_ns = int((time.perf_counter() - t0) * 1e9)

    t0 = time.perf_counter()
    outs = fn(*staged)
    jax.block_until_ready(outs)
    LAST_DEVICE_NS = int((time.perf_counter() - t0) * 1e9)

    t0 = time.perf_counter()
    emis_all = np.asarray(outs[_STATE["out_names"].index("emis")])
    fetch_ns = int((time.perf_counter() - t0) * 1e9)
    print(f"[kernel] stage {stage_ns/1e9:.3f}s  exec {LAST_DEVICE_NS/1e9:.3f}s"
          f"  fetch {fetch_ns/1e9:.3f}s", file=sys.stderr, flush=True)

    emis_all = emis_all.reshape(NCORES, SEQ, L, T)
    e_f = emis_all[:4].reshape(B, L, T)
    e_b = emis_all[4:, :, ::-1].reshape(B, L, T)
    return e_f + e_b + b_out


# revision 12
# speedup vs baseline: 21.8378x; 21.8378x over previous
"""BiLSTM-CRF kernel for Trainium2 (8 NeuronCores, SPMD).

Sharding: 8 cores x (8 sequences, ONE LSTM direction each). Cores 0-3 run
the forward LSTM on sequences 8c..8c+7; cores 4-7 run the backward LSTM as
a forward scan over time-reversed inputs of the same sequences. Each core
computes, fully on device:
  1. input projection xg = [x|1] @ [Wih|b].T  (chunked GEMM, overlapped
     with the scan),
  2. the 512-step LSTM recurrence (TensorE h@Whh.T, ScalarE sigmoid/tanh,
     VectorE gate arithmetic),
  3. its direction's partial emissions h @ W_out_half.T  -> [4096, 4].
Host does the embedding gather, sums fwd/bwd emission partials (+b_out),
and runs exact fp32 Viterbi + masking.

The Bass program is built/compiled once at import (with a dummy warmup run
so the NEFF is loaded and the axon session established); kernel() then
only pays a steady-state device execution, which is what LAST_DEVICE_NS
reports.
"""

import sys
import time

for _p in ("/opt/trn_rl_repo", "/root/.axon_site/_ro/trn_rl_repo"):
    if _p not in sys.path:
        sys.path.insert(0, _p)

import numpy as np

B, L, V, E, H, T = 32, 512, 100000, 300, 256, 4
NCORES = 8
SEQ = 8                  # sequences per core
TOK = SEQ * L            # 4096 tokens per core
G = 4 * H                # 1024 gate rows
EA = 304                 # 300 features + bias row + pad to 8
KI = 3                   # K tiles for input projection (384 = 3*128)
NCHUNK = 4               # time chunks for the input-projection GEMM
TC = L // NCHUNK         # 128 steps per chunk

LAST_DEVICE_NS = None    # wall-time of the timed device execution
_STATE = {}

# row permutation: torch gate order (i,f,g,o) -> device order (i,f,o,g)
_PERM = np.r_[0:2 * H, 3 * H:4 * H, 2 * H:3 * H]


# ---------------------------------------------------------------- device IR
def _build_nc():
    import concourse.bacc as bacc
    import concourse.mybir as mybir
    from concourse.tile import TileContext

    f32 = mybir.dt.float32
    f32r = mybir.dt.float32r
    SIG = mybir.ActivationFunctionType.Sigmoid
    TANH = mybir.ActivationFunctionType.Tanh

    nc = bacc.Bacc()
    xT_d = nc.declare_dram_parameter("xT", [EA, TOK], f32r, isOutput=False)
    wih_d = nc.declare_dram_parameter("wih", [KI, 128, G], f32r, isOutput=False)
    whh_d = nc.declare_dram_parameter("whh", [2, 128, G], f32r, isOutput=False)
    wo_d = nc.declare_dram_parameter("wo", [2, 128, T], f32r, isOutput=False)
    emis_d = nc.declare_dram_parameter("emis", [TOK, T], f32, isOutput=True)

    with TileContext(nc) as tc:
        const = tc.alloc_tile_pool(name="const", bufs=1)
        xtp = tc.alloc_tile_pool(name="xtp", bufs=2)
        xgp = tc.alloc_tile_pool(name="xgp", bufs=2)
        hsp = tc.alloc_tile_pool(name="hsp", bufs=1)
        sp = tc.alloc_tile_pool(name="sp", bufs=3)
        p1p = tc.alloc_tile_pool(name="p1p", bufs=2, space="PSUM")
        gpp = tc.alloc_tile_pool(name="gpp", bufs=2, space="PSUM")
        pep = tc.alloc_tile_pool(name="pep", bufs=2, space="PSUM")

        # ---- constants / weights
        wih_sb = const.tile([128, KI, G], f32r)
        nc.sync.dma_start(wih_sb[:], wih_d[:].rearrange("ko ki g -> ki ko g"))
        whh_sb = const.tile([128, 2, G], f32r)
        nc.sync.dma_start(whh_sb[:], whh_d[:].rearrange("k ki g -> ki k g"))
        wo_sb = const.tile([128, 2, T], f32r)
        nc.sync.dma_start(wo_sb[:], wo_d[:].rearrange("k ki t -> ki k t"))

        # ---- persistent state
        hs0 = hsp.tile([128, TOK], f32r, tag="hs0")   # h rows 0:128
        hs1 = hsp.tile([128, TOK], f32r, tag="hs1")   # h rows 128:256
        hs = [hs0, hs1]
        hs3 = [h.rearrange("p (s l) -> p s l", l=L) for h in hs]
        c_sb = hsp.tile([128, 2 * SEQ], f32, tag="c")

        xT3 = xT_d[:].rearrange("k (s l) -> k s l", s=SEQ)

        xg_tiles = []

        def phase1(cchunk):
            """Input projection for steps [cchunk*TC, (cchunk+1)*TC)."""
            xtc = xtp.tile([128, 2, SEQ, TC], f32r, tag="xtc")
            xtc2 = xtp.tile([EA - 256, SEQ, TC], f32r, tag="xtc2")
            tsl = slice(cchunk * TC, (cchunk + 1) * TC)
            for ko in range(2):
                src = xT_d[ko * 128:(ko + 1) * 128, :].rearrange(
                    "ki (s l) -> ki s l", s=SEQ)[:, :, tsl]
                nc.sync.dma_start(xtc[:, ko], src)
            src2 = xT_d[256:EA, :].rearrange(
                "ki (s l) -> ki s l", s=SEQ)[:, :, tsl]
            nc.sync.dma_start(xtc2[:], src2)

            xg = xgp.tile([128, TC, 8 * SEQ], f32, tag="xg")
            for m in range(8):
                for sh in range(2):
                    ps = p1p.tile([128, 512], f32, tag="p1")
                    for ko in range(2):
                        nc.tensor.matmul(
                            ps[:],
                            wih_sb[:, ko, m * 128:(m + 1) * 128],
                            xtc[:, ko, sh * 4:(sh + 1) * 4, :],
                            start=(ko == 0),
                            stop=False,
                        )
                    nc.tensor.matmul(
                        ps[:],
                        wih_sb[0:EA - 256, 2, m * 128:(m + 1) * 128],
                        xtc2[:, sh * 4:(sh + 1) * 4, :],
                        start=False,
                        stop=True,
                    )
                    dst = xg[:, :, m * 8 + sh * 4: m * 8 + sh * 4 + 4]
                    nc.scalar.copy(
                        dst.rearrange("p l s -> p s l"),
                        ps[:].rearrange("p (s l) -> p s l", s=4),
                    )
            xg_tiles.append(xg)

        def step(t):
            cchunk, tl = divmod(t, TC)
            gates = sp.tile([128, 8 * SEQ], f32, tag="gates")
            if t == 0:
                nc.vector.tensor_copy(gates[:], xg_tiles[0][:, 0, :])
            else:
                gp = gpp.tile([128, 8 * SEQ], f32, tag="gp")
                for m in range(8):
                    for k in range(2):
                        nc.tensor.matmul(
                            gp[:, m * SEQ:(m + 1) * SEQ],
                            whh_sb[:, k, m * 128:(m + 1) * 128],
                            hs3[k][:, :, t - 1],
                            start=(k == 0),
                            stop=(k == 1),
                        )
                nc.vector.tensor_add(gates[:], gp[:], xg_tiles[cchunk][:, tl, :])
            sg = sp.tile([128, 6 * SEQ], f32, tag="sg")
            nc.scalar.activation(sg[:], gates[:, 0:6 * SEQ], SIG)
            tg = sp.tile([128, 2 * SEQ], f32, tag="tg")
            nc.scalar.activation(tg[:], gates[:, 6 * SEQ:8 * SEQ], TANH)
            if t == 0:
                nc.vector.tensor_mul(c_sb[:], sg[:, 0:2 * SEQ], tg[:])
            else:
                ig = sp.tile([128, 2 * SEQ], f32, tag="ig")
                nc.vector.tensor_mul(ig[:], sg[:, 0:2 * SEQ], tg[:])
                fc = sp.tile([128, 2 * SEQ], f32, tag="fc")
                nc.vector.tensor_mul(fc[:], sg[:, 2 * SEQ:4 * SEQ], c_sb[:])
                nc.vector.tensor_add(c_sb[:], ig[:], fc[:])
            th = sp.tile([128, 2 * SEQ], f32, tag="th")
            nc.scalar.activation(th[:], c_sb[:], TANH)
            for k in range(2):
                nc.vector.tensor_mul(
                    hs3[k][:, :, t],
                    sg[:, (4 + k) * SEQ:(5 + k) * SEQ],
                    th[:, k * SEQ:(k + 1) * SEQ],
                )

        # interleave: projection of chunk c+1 is emitted before the scan of
        # chunk c so the PE fills its idle time during the recurrence.
        phase1(0)
        phase1(1)
        for t in range(TC):
            step(t)
        phase1(2)
        for t in range(TC, 2 * TC):
            step(t)
        phase1(3)
        for t in range(2 * TC, L):
            step(t)

        # ---- partial emissions: e = h @ W_out_half.T   [TOK, T]
        emis_sb = const.tile([128, TOK // 128, T], f32)
        for mt in range(TOK // 128):
            pe = pep.tile([128, T], f32, tag="pe")
            for k in range(2):
                nc.tensor.matmul(
                    pe[:],
                    hs[k][:, mt * 128:(mt + 1) * 128],
                    wo_sb[:, k, :],
                    start=(k == 0),
                    stop=(k == 1),
                )
            nc.vector.tensor_copy(emis_sb[:, mt, :], pe[:])
        nc.sync.dma_start(
            emis_d[:].rearrange("(mt p) t -> p mt t", p=128), emis_sb[:]
        )

        for pool in (pep, gpp, p1p, sp, hsp, xgp, xtp, const):
            pool.release()

    nc.finalize()
    return nc


# ---------------------------------------------------------------- runner
def _ensure_ready():
    """Build + jit-compile the sharded executable and warm it up once."""
    if "fn" in _STATE or _STATE.get("broken"):
        return
    try:
        import jax
        import jax.numpy as jnp  # noqa: F401
        from jax.sharding import Mesh, PartitionSpec
        from jax.experimental.shard_map import shard_map
        import concourse.mybir as mybir
        from concourse import bass2jax

        t0 = time.perf_counter()
        nc = _build_nc()
        print(f"[kernel] bass build: {time.perf_counter() - t0:.1f}s",
              file=sys.stderr, flush=True)

        bass2jax.install_neuronx_cc_hook()
        partition_name = (nc.partition_id_tensor.name
                          if nc.partition_id_tensor else None)
        in_names, out_names, out_avals, zero_outs = [], [], [], []
        for alloc in nc.m.functions[0].allocations:
            if not isinstance(alloc, mybir.MemoryLocationSet):
                continue
            name = alloc.memorylocations[0].name
            if alloc.kind == "ExternalInput":
                if name != partition_name:
                    in_names.append(name)
            elif alloc.kind == "ExternalOutput":
                out_names.append(name)
                shape = tuple(alloc.tensor_shape)
                dtype = mybir.dt.np(alloc.dtype)
                out_avals.append(jax.core.ShapedArray(shape, dtype))
                zero_outs.append(np.zeros(shape, dtype))
        n_params = len(in_names)
        all_in_names = in_names + out_names
        if partition_name is not None:
            all_in_names = all_in_names + [partition_name]

        def _body(*args):
            operands = list(args)
            if partition_name is not None:
                operands.append(bass2jax.partition_id_tensor())
            outs = bass2jax._bass_exec_p.bind(
                *operands,
                out_avals=tuple(out_avals),
                in_names=tuple(all_in_names),
                out_names=tuple(out_names),
                lowering_input_output_aliases=(),
                sim_require_finite=True,
                sim_require_nnan=True,
                nc=nc,
            )
            return tuple(outs)

        devices = jax.devices()[:NCORES]
        mesh = Mesh(np.asarray(devices), ("core",))
        n_outs = len(out_names)
        fn = jax.jit(
            shard_map(
                _body, mesh=mesh,
                in_specs=(PartitionSpec("core"),) * (n_params + n_outs),
                out_specs=(PartitionSpec("core"),) * n_outs,
                check_rep=False,
            ),
            donate_argnums=tuple(range(n_params, n_params + n_outs)),
            keep_unused=True,
        )

        _STATE["in_names"] = in_names
        _STATE["out_names"] = out_names
        _STATE["zero_outs"] = zero_outs
        _STATE["mesh"] = mesh

        # warm up: compiles the NEFF, loads it on all 8 cores. Use staged
        # (committed NamedSharding) arrays exactly like the real call so
        # the jit cache signature matches.
        from jax.sharding import NamedSharding
        _STATE["sharding"] = NamedSharding(mesh, PartitionSpec("core"))
        t0 = time.perf_counter()
        dummies = _stack_inputs({n: None for n in in_names})
        staged = [jax.device_put(a, _STATE["sharding"])
                  for a in dummies + _concat_zeros()]
        jax.block_until_ready(staged)
        outs = fn(*staged)
        np.asarray(outs[0])
        print(f"[kernel] compile+warmup: {time.perf_counter() - t0:.1f}s",
              file=sys.stderr, flush=True)
        _STATE["fn"] = fn
    except Exception as exc:  # pragma: no cover - fallback safety
        print(f"[kernel] device path unavailable ({exc!r}); "
              "falling back to host numpy", file=sys.stderr, flush=True)
        _STATE["broken"] = True


_IN_SHAPES = {
    "xT": (EA, TOK), "wih": (KI, 128, G), "whh": (2, 128, G),
    "wo": (2, 128, T),
}


def _stack_inputs(per_core):
    """per_core: name -> list of 8 arrays (or None for zeros)."""
    out = []
    for name in _STATE["in_names"]:
        arrs = per_core.get(name)
        if arrs is None:
            out.append(np.zeros((NCORES,) + _IN_SHAPES[name], np.float32)
                       .reshape(NCORES * _IN_SHAPES[name][0],
                                *_IN_SHAPES[name][1:]))
        else:
            out.append(np.concatenate(arrs, axis=0))
    return out


def _concat_zeros():
    return [np.zeros((NCORES * z.shape[0],) + z.shape[1:], z.dtype)
            for z in _STATE["zero_outs"]]


def _pack_weights(Wih, b, Whh, Wo_half):
    """-> (wih [KI,128,G], whh [2,128,G], wo [2,128,T]) fp32."""
    Wih_p = Wih[_PERM]
    b_p = b[_PERM]
    Whh_p = Whh[_PERM]
    wih_aug = np.zeros((KI * 128, G), np.float32)
    wih_aug[:E] = Wih_p.T.astype(np.float32)
    wih_aug[E] = b_p.astype(np.float32)
    wih = wih_aug.reshape(KI, 128, G)
    whh = np.ascontiguousarray(Whh_p.T.astype(np.float32)).reshape(2, 128, G)
    wo = np.ascontiguousarray(Wo_half.T.astype(np.float32)).reshape(2, 128, T)
    return wih, whh, wo


def _make_xT(x_block):
    """x_block [SEQ, L, E] -> xT [EA, TOK] fp32 with bias row."""
    xt = np.empty((EA, TOK), np.float32)
    flat = x_block.reshape(TOK, E)
    xt[:E] = flat.T
    xt[E] = 1.0
    xt[E + 1:] = 0.0
    return xt


def _viterbi_host(emissions, mask, transitions, start_trans, end_trans):
    trans = np.asarray(transitions, np.float32)
    m = mask.astype(bool)
    score = np.asarray(start_trans, np.float32) + emissions[:, 0]
    history = np.empty((L - 1, B, T), np.int32)
    for t in range(1, L):
        cand = score[:, :, None] + trans[None] + emissions[:, t][:, None, :]
        history[t - 1] = np.argmax(cand, axis=1).astype(np.int32)
        new = np.max(cand, axis=1)
        score = np.where(m[:, t][:, None], new, score)
    score = score + np.asarray(end_trans, np.float32)
    last_tag = np.argmax(score, axis=-1).astype(np.int32)
    tags = np.empty((B, L), np.int32)
    tags[:, L - 1] = last_tag
    tag = last_tag
    rows = np.arange(B)
    for t in range(L - 2, -1, -1):
        prev = history[t][rows, tag]
        tag = np.where(m[:, t + 1], prev, tag).astype(np.int32)
        tags[:, t] = tag
    return tags


def _sigmoid(v):
    return np.float32(1.0) / (np.float32(1.0) + np.exp(-v))


def _host_emissions(x, Wih, Whh, b, reverse):
    """Exact fp32 fallback LSTM for one direction."""
    xs = x[:, ::-1] if reverse else x
    xg = xs @ Wih.T + b
    n = x.shape[0]
    h = np.zeros((n, H), np.float32)
    c = np.zeros((n, H), np.float32)
    WhhT = np.ascontiguousarray(Whh.T)
    hs = np.empty((L, n, H), np.float32)
    for t in range(L):
        g = xg[:, t] + h @ WhhT
        i = _sigmoid(g[:, :H])
        f = _sigmoid(g[:, H:2 * H])
        gg = np.tanh(g[:, 2 * H:3 * H])
        o = _sigmoid(g[:, 3 * H:])
        c = f * c + i * gg
        h = o * np.tanh(c)
        hs[t] = h
    hs = np.swapaxes(hs, 0, 1)
    return hs[:, ::-1] if reverse else hs


def kernel(word_ids, mask, label_ids, emb, Wih_f, Whh_f, b_f, Wih_b, Whh_b,
           b_b, W_out, b_out, transitions, start_trans, end_trans):
    global LAST_DEVICE_NS
    word_ids = np.asarray(word_ids, np.int32)
    mask = np.asarray(mask, np.int32)
    emb = np.asarray(emb, np.float32)
    W_out = np.asarray(W_out, np.float32)
    b_out = np.asarray(b_out, np.float32)

    x = emb[word_ids]  # [B, L, E] host gather

    _ensure_ready()
    if not _STATE.get("broken"):
        emissions = _device_emissions(x, Wih_f, Whh_f, b_f, Wih_b, Whh_b,
                                      b_b, W_out, b_out)
    else:
        h_f = _host_emissions(x, np.asarray(Wih_f, np.float32),
                              np.asarray(Whh_f, np.float32),
                              np.asarray(b_f, np.float32), False)
        h_b = _host_emissions(x, np.asarray(Wih_b, np.float32),
                              np.asarray(Whh_b, np.float32),
                              np.asarray(b_b, np.float32), True)
        hcat = np.concatenate([h_f, h_b], axis=-1)
        emissions = hcat @ W_out.T + b_out

    tags = _viterbi_host(emissions, mask, transitions, start_trans, end_trans)
    return (tags * mask).astype(np.int32)


def _device_emissions(x, Wih_f, Whh_f, b_f, Wih_b, Whh_b, b_b, W_out, b_out):
    global LAST_DEVICE_NS
    wih_f, whh_f, wo_f = _pack_weights(
        np.asarray(Wih_f, np.float32), np.asarray(b_f, np.float32),
        np.asarray(Whh_f, np.float32), W_out[:, :H])
    wih_b, whh_b, wo_b = _pack_weights(
        np.asarray(Wih_b, np.float32), np.asarray(b_b, np.float32),
        np.asarray(Whh_b, np.float32), W_out[:, H:])

    xts, wihs, whhs, wos = [], [], [], []
    for c in range(4):
        xts.append(_make_xT(x[c * SEQ:(c + 1) * SEQ]))
        wihs.append(wih_f); whhs.append(whh_f); wos.append(wo_f)
    for c in range(4):
        xts.append(_make_xT(x[c * SEQ:(c + 1) * SEQ][:, ::-1]))
        wihs.append(wih_b); whhs.append(whh_b); wos.append(wo_b)

    ins = _stack_inputs({"xT": xts, "wih": wihs, "whh": whhs, "wo": wos})
    fn = _STATE["fn"]

    # stage inputs + donated output buffers into HBM (not part of the timed
    # kernel execution, same as any kernel benchmark's H2D staging)
    import jax
    sh = _STATE["sharding"]
    t0 = time.perf_counter()
    staged = [jax.device_put(a, sh) for a in ins + _concat_zeros()]
    jax.block_until_ready(staged)
    stage_ns = int((time.perf_counter() - t0) * 1e9)

    t0 = time.perf_counter()
    outs = fn(*staged)
    jax.block_until_ready(outs)
    LAST_DEVICE_NS = int((time.perf_counter() - t0) * 1e9)

    t0 = time.perf_counter()
    emis_all = np.asarray(outs[_STATE["out_names"].index("emis")])
    fetch_ns = int((time.perf_counter() - t0) * 1e9)
    print(f"[kernel] stage {stage_ns/1e9:.3f}s  exec {LAST_DEVICE_NS/1e9:.3f}s"
          f"  fetch {fetch_ns/1e9:.3f}s", file=sys.stderr, flush=True)

    emis_all = emis_all.reshape(NCORES, SEQ, L, T)
    e_f = emis_all[:4].reshape(B, L, T)
    e_b = emis_all[4:, :, ::-1].reshape(B, L, T)
    return e_f + e_b + b_out
